# revision 7
# baseline (speedup 1.0000x reference)
"""Trainium2 Bass kernel for the GCN model (nn_GCNModel_57853209477141).

Model: 3x GCNConv(128->128, sym-norm with self loops) with ReLU, question
embedding MLP, concat, 2-layer MLP head -> [50000, 32].

Strategy (8 NeuronCores, single SPMD launch):
- dst-node sharding: global tiles of 128 nodes; snake-dealt across cores by
  edge count so one compile-time chunk schedule serves all 8 cores.
- GCN norm factorization: agg[v] = dinv[v] * sum_{e->v} (dinv*h)[src_e]; the
  per-edge norm disappears by storing h~ = dinv*h in the gather table.
- layer-0 table is computed REPLICATED on every core (x is replicated), so
  no AllGather is needed for it; layers 1/2 tables are AllGathered per
  7-slot block from the production epilogues (overlapped with compute).
- TRANSPOSED aggregation: psT[feat, dst] += glo_chunk.T @ onehot_chunk, so
  the epilogue produces h^T directly (no PE transposes anywhere), the layer
  bias is a native per-partition activation bias, and dinv[dst] is applied
  with one DVE multiply against a host-precomputed broadcast plane.
- gathers: ONE large bf16 dma_gather per (4-slot group x table-half)
  (int16 gather indices address at most 32768 rows -> table split at row
  32768), prefetched 2 groups ahead, round-robin over 4 SWDGE queues.
- question path: qq = relu(qe@fc0+b)@fc1[128:] kept on-chip; added in the
  head via a one-hot-over-graphs matmul (sel plane precomputed on host).
- head: u^T = fc1a.T @ h3^T + qq.T @ sel; out = relu(u)^T @ fc2 + fc2_b.

Host preprocessing is index work only (sharding, edge sort, index planes);
all O(E*F) / O(N*F*F) float work runs on device.
"""
import os
import sys
import types
from contextlib import ExitStack

import numpy as np

# ---------------------------------------------------------------- constants
N = 50000
E = 800000
G = 64
P = 128
NCORES = 8
TPC = 49  # tile slots per core
SPB = 7  # slots per AllGather block
NBLK = 7
SLOT_ROWS = TPC * P  # 6272
NT = NCORES * SLOT_ROWS  # 50176
NPOS = NT // P  # 392 global table tile positions
HALF = 32768  # int16 gather index limit -> table split row
QD = 768
OUTC = 32
GROUP = 4  # slots per gather group / head group

BF16 = np.dtype("bfloat16")


def _install_axon_prof():
    """Register NTFF profile hook if the image's antenv lacks it; neuter
    bucket upload (zero-egress). Harmless when running without tracing."""
    try:
        from antenv import axon_hooks  # noqa: F401
    except ImportError:
        try:
            import antenv
            from trn_agent_boot.trn_boot import _ntff_profile_via_ctypes

            hook = _ntff_profile_via_ctypes("/opt/axon/libaxon_pjrt.so")
            mod = types.ModuleType("antenv.axon_hooks")
            mod.get_axon_ntff_profile_hook = lambda: hook
            mod.set_axon_ntff_profile_hook = lambda h: None
            sys.modules["antenv.axon_hooks"] = mod
            antenv.axon_hooks = mod
        except Exception:
            pass
    try:
        import concourse.bass_utils as bu

        bu.upload_artifacts = lambda tmpdir: "local://" + str(tmpdir)
    except Exception:
        pass


def _wrap16(arr):
    """int array -> [128, len/16] int16 plane (idx i at partition i%16,
    col i//16; replicated to all 8 gpsimd core groups)."""
    m = np.asarray(arr, dtype=np.int16).reshape(-1, 16).T
    return np.tile(m, (8, 1))


# ---------------------------------------------------------------- host prep
def preprocess(edge_index, batch):
    src = np.asarray(edge_index[0], dtype=np.int64)
    dst = np.asarray(edge_index[1], dtype=np.int64)
    deg = (np.bincount(dst, minlength=N) + 1).astype(np.float64)
    dinv = (1.0 / np.sqrt(deg)).astype(np.float32)

    n_tiles = (N + P - 1) // P  # 391
    tile_of_node = np.arange(N) // P
    dst_tile = dst // P
    tile_counts = np.bincount(dst_tile, minlength=n_tiles)

    # snake-deal tiles (sorted by edge count desc) across cores
    order_all = np.argsort(-tile_counts, kind="stable")
    core_tiles = [[] for _ in range(NCORES)]
    for r in range(TPC):
        batch_t = order_all[r * NCORES : (r + 1) * NCORES]
        seq = range(NCORES) if r % 2 == 0 else range(NCORES - 1, -1, -1)
        for j, c in enumerate(seq):
            core_tiles[c].append(int(batch_t[j]) if j < len(batch_t) else -1)

    core_of_tile = np.full(n_tiles, -1, dtype=np.int64)
    slot_of_tile = np.full(n_tiles, -1, dtype=np.int64)
    for c in range(NCORES):
        for s, t in enumerate(core_tiles[c]):
            if t >= 0:
                core_of_tile[t] = c
                slot_of_tile[t] = s

    # block-major table row for every node (same layout for all 3 layers)
    blk = slot_of_tile[tile_of_node] // SPB
    table_row = (
        blk * (NCORES * SPB * P)
        + core_of_tile[tile_of_node] * (SPB * P)
        + (slot_of_tile[tile_of_node] % SPB) * P
        + (np.arange(N) % P)
    )

    order = np.argsort(dst_tile, kind="stable")
    src_sorted = src[order]
    dst_sorted = dst[order]
    sorted_tiles = dst_tile[order]
    tile_starts = np.searchsorted(sorted_tiles, np.arange(n_tiles))
    tile_ends = np.searchsorted(sorted_tiles, np.arange(n_tiles), side="right")

    src_rows = table_row[src_sorted]
    is_lo = src_rows < HALF

    # per-(core, slot, half) edge lists + common chunk schedule
    cnt = np.zeros((NCORES, TPC, 2), dtype=np.int64)
    elists = [[None] * TPC for _ in range(NCORES)]  # (rows_lo, din_lo, rows_hi, din_hi)
    for c in range(NCORES):
        for s in range(TPC):
            t = core_tiles[c][s]
            if t < 0:
                elists[c][s] = (
                    np.zeros(0, np.int64), np.zeros(0, np.int64),
                    np.zeros(0, np.int64), np.zeros(0, np.int64),
                )
                continue
            lo_, hi_ = tile_starts[t], tile_ends[t]
            rows = src_rows[lo_:hi_]
            din = dst_sorted[lo_:hi_] % P
            m = is_lo[lo_:hi_]
            elists[c][s] = (rows[m], din[m], rows[~m] - HALF, din[~m])
            cnt[c, s, 0] = int(m.sum())
            cnt[c, s, 1] = int((~m).sum())

    nch = np.ceil(cnt.max(axis=0) / P).astype(np.int64)  # [TPC, 2]

    # gather-call schedule: chunk columns [grp lo (slot-major) | grp hi]
    groups = [list(range(g, min(g + GROUP, TPC))) for g in range(0, TPC, GROUP)]
    slot_base = np.zeros((TPC, 2), dtype=np.int64)  # gather-col base per slot
    grp_info = []  # (slots, col0, lo_tot, hi_tot)
    cur = 0
    for gs in groups:
        col0 = cur
        for s in gs:
            slot_base[s, 0] = cur
            cur += nch[s, 0]
        for s in gs:
            slot_base[s, 1] = cur
            cur += nch[s, 1]
        lo_tot = int(nch[gs, 0].sum())
        hi_tot = int(nch[gs, 1].sum())
        grp_info.append((gs, col0, lo_tot, hi_tot))
    TCH = int(cur)

    # dstin plane is SLOT-major: [slot: lo chunks | hi chunks] so one DVE
    # is_equal per slot builds the whole one-hot for that slot.
    dcol = np.zeros((TPC, 2), dtype=np.int64)
    cur = 0
    for s in range(TPC):
        dcol[s, 0] = cur
        cur += nch[s, 0]
        dcol[s, 1] = cur
        cur += nch[s, 1]
    assert cur == TCH

    idx_T = np.zeros((NCORES, 128, TCH * 8), dtype=np.int16)
    dstin_T = np.full((NCORES, 128, TCH), -1.0, dtype=np.float32)
    for c in range(NCORES):
        for s in range(TPC):
            rows_lo, din_lo, rows_hi, din_hi = elists[c][s]
            for h, (rows, din) in enumerate(((rows_lo, din_lo), (rows_hi, din_hi))):
                nchunks = int(nch[s, h])
                if nchunks == 0:
                    continue
                pad = nchunks * P
                rbuf = np.zeros(pad, dtype=np.int64)
                rbuf[: len(rows)] = rows
                dbuf = np.full(pad, -1.0, dtype=np.float32)
                dbuf[: len(din)] = din.astype(np.float32)
                b = int(slot_base[s, h])
                idx_T[c, :, b * 8 : (b + nchunks) * 8] = _wrap16(rbuf)
                db = int(dcol[s, h])
                dstin_T[c, :, db : db + nchunks] = dbuf.reshape(nchunks, P).T
    del elists

    # per-core planes: dinv per slot (per-partition), dinv broadcast across
    # partitions (free-dim scale in transposed epilogue), graph-select
    # one-hot, node permutation
    dinv_slot = np.zeros((NCORES, P, TPC), dtype=np.float32)
    dinvB = np.zeros((NCORES, P, SLOT_ROWS), dtype=np.float32)
    sel = np.zeros((NCORES, G, SLOT_ROWS), dtype=np.float32)
    node_perm = np.full((NCORES, SLOT_ROWS), -1, dtype=np.int64)
    batch = np.asarray(batch, dtype=np.int64)
    for c in range(NCORES):
        for s in range(TPC):
            t = core_tiles[c][s]
            if t < 0:
                continue
            v0 = t * P
            v1 = min(v0 + P, N)
            n = v1 - v0
            dinv_slot[c, :n, s] = dinv[v0:v1]
            dinvB[c, :, s * P : s * P + n] = dinv[v0:v1][None, :]
            sel[c, batch[v0:v1], s * P + np.arange(n)] = 1.0
            node_perm[c, s * P : s * P + n] = np.arange(v0, v1)

    # replicated layer-0 production: x columns permuted to table order
    pos_node = np.full(NT, -1, dtype=np.int64)
    pos_node[table_row] = np.arange(N)
    dinv_perm = np.zeros((P, NPOS), dtype=np.float32)
    valid = pos_node >= 0
    dp = np.zeros(NT, dtype=np.float32)
    dp[valid] = dinv[pos_node[valid]]
    dinv_perm[:, :] = dp.reshape(NPOS, P).T

    return dict(
        nch=nch,
        dcol=dcol,
        grp_info=grp_info,
        slot_base=slot_base,
        TCH=TCH,
        idx_T=idx_T,
        dstin_T=dstin_T,
        dinv_slot=dinv_slot,
        dinvB=dinvB,
        sel=sel,
        node_perm=node_perm,
        pos_node=pos_node,
        dinv_perm=dinv_perm,
    )


# ------------------------------------------------------------- bass program
def build_program(nch, dcol, grp_info, slot_base, TCH):
    import concourse.bacc as bacc
    import concourse.bass as bass
    import concourse.tile as tile
    from concourse import library_config, mybir
    from concourse.masks import make_identity

    FDT = mybir.dt.bfloat16
    F32 = mybir.dt.float32
    I16 = mybir.dt.int16

    LOMAX = max(lo for _, _, lo, _ in grp_info)
    HIMAX = max(hi for _, _, _, hi in grp_info)
    NTOTMAX = int((nch[:, 0] + nch[:, 1]).max())

    nc = bacc.Bacc("TRN2", target_bir_lowering=False, num_swdge_queues=4)
    dp = nc.declare_dram_parameter
    xT_perm = dp("xT_perm", [P, NT], FDT, isOutput=False)  # replicated, table order
    xT_own = dp("xT_own", [P, SLOT_ROWS], FDT, isOutput=False)  # own slots
    idx_in = dp("idx_in", [P, TCH * 8], I16, isOutput=False)
    dstin = dp("dstin", [P, TCH], FDT, isOutput=False)
    iota_in = dp("iota_in", [P, NTOTMAX * P], FDT, isOutput=False)
    dinv_in = dp("dinv_in", [P, TPC], F32, isOutput=False)
    dinvB_in = dp("dinvB_in", [P, SLOT_ROWS], F32, isOutput=False)
    dinv_perm_in = dp("dinv_perm", [P, NPOS], F32, isOutput=False)
    sel_in = dp("sel_in", [G, SLOT_ROWS], FDT, isOutput=False)
    W_in = [dp(f"W{i}", [P, P], FDT, isOutput=False) for i in range(3)]
    brow_in = dp("brow", [P, 3], F32, isOutput=False)
    qeT_in = dp("qeT", [QD, G], F32, isOutput=False)
    fc0w_in = dp("fc0w", [QD, P], F32, isOutput=False)
    fc0bb_in = dp("fc0bb", [P, P], F32, isOutput=False)
    fc1a_in = dp("fc1a", [P, P], FDT, isOutput=False)
    fc1b_in = dp("fc1b", [P, P], F32, isOutput=False)
    fc1bcol_in = dp("fc1bcol", [P, 1], F32, isOutput=False)
    fc2w_in = dp("fc2w", [P, OUTC], FDT, isOutput=False)
    fc2bb_in = dp("fc2bb", [P, OUTC], F32, isOutput=False)
    out_d = dp("out", [SLOT_ROWS, OUTC], F32, isOutput=True)

    cc_in = nc.dram_tensor("cc_in", [SLOT_ROWS, P], FDT)
    table0 = nc.dram_tensor("table0", [NT, P], FDT)
    tables = [
        table0,
        nc.dram_tensor("table1", [NT, P], FDT, addr_space="Shared"),
        nc.dram_tensor("table2", [NT, P], FDT, addr_space="Shared"),
    ]

    with tile.TileContext(nc) as tc, ExitStack() as ctx:
        nc.gpsimd.load_library(library_config.mlp)

        const = ctx.enter_context(tc.tile_pool(name="const", bufs=1))
        xp = ctx.enter_context(tc.tile_pool(name="xp", bufs=3))
        l0p = ctx.enter_context(tc.tile_pool(name="l0p", bufs=4))
        gp = ctx.enter_context(tc.tile_pool(name="gp", bufs=3))
        ghp = ctx.enter_context(tc.tile_pool(name="ghp", bufs=3))
        ohp = ctx.enter_context(tc.tile_pool(name="ohp", bufs=4))
        htp = ctx.enter_context(tc.tile_pool(name="htp", bufs=3))
        psagg = ctx.enter_context(tc.tile_pool(name="psagg", bufs=2, space="PSUM"))
        psp = ctx.enter_context(tc.tile_pool(name="psp", bufs=3, space="PSUM"))
        psh = ctx.enter_context(tc.tile_pool(name="psh", bufs=2, space="PSUM"))
        epi = ctx.enter_context(tc.tile_pool(name="epi", bufs=4))

        # ---- constants
        idx_sb = const.tile([P, TCH * 8], I16)
        nc.scalar.dma_start(out=idx_sb[:], in_=idx_in[:])
        dstin_sb = const.tile([P, TCH], FDT)
        nc.scalar.dma_start(out=dstin_sb[:], in_=dstin[:])
        iota_sb = const.tile([P, NTOTMAX * P], FDT)
        nc.scalar.dma_start(out=iota_sb[:], in_=iota_in[:])
        dinv_sb = const.tile([P, TPC], F32)
        nc.sync.dma_start(out=dinv_sb[:], in_=dinv_in[:])
        dinvB_sb = const.tile([P, SLOT_ROWS], F32)
        nc.sync.dma_start(out=dinvB_sb[:], in_=dinvB_in[:])
        dinv_perm_sb = const.tile([P, NPOS], F32)
        nc.sync.dma_start(out=dinv_perm_sb[:], in_=dinv_perm_in[:])
        sel_sb = const.tile([G, SLOT_ROWS], FDT)
        nc.sync.dma_start(out=sel_sb[:], in_=sel_in[:])
        W_sb = []
        for i in range(3):
            w = const.tile([P, P], FDT, tag=f"W{i}")
            nc.sync.dma_start(out=w[:], in_=W_in[i][:])
            W_sb.append(w)
        brow_sb = const.tile([P, 3], F32)
        nc.sync.dma_start(out=brow_sb[:], in_=brow_in[:])
        fc1a_sb = const.tile([P, P], FDT)
        nc.sync.dma_start(out=fc1a_sb[:], in_=fc1a_in[:])
        fc1bcol_sb = const.tile([P, 1], F32)
        nc.sync.dma_start(out=fc1bcol_sb[:], in_=fc1bcol_in[:])
        fc2w_sb = const.tile([P, OUTC], FDT)
        nc.sync.dma_start(out=fc2w_sb[:], in_=fc2w_in[:])
        fc2bb_sb = const.tile([P, OUTC], F32)
        nc.sync.dma_start(out=fc2bb_sb[:], in_=fc2bb_in[:])
        ident = const.tile([P, P], F32)
        make_identity(nc, ident[:])
        ident_r = const.tile([P, P], FDT, tag="ident_r")
        nc.vector.tensor_copy(out=ident_r[:], in_=ident[:])

        # ---- question path: qq = relu(qe@fc0+fc0_b)@fc1b  (bf16, on-chip;
        # fc1_b enters later as the head activation bias)
        qe_sb = const.tile([P, 6 * G], F32)
        fc0w_sb = const.tile([P, 6 * P], F32)
        for k in range(6):
            nc.sync.dma_start(
                out=qe_sb[:, k * G : (k + 1) * G], in_=qeT_in[k * P : (k + 1) * P, :]
            )
            nc.sync.dma_start(
                out=fc0w_sb[:, k * P : (k + 1) * P],
                in_=fc0w_in[k * P : (k + 1) * P, :],
            )
        fc0bb_sb = const.tile([P, P], F32)
        nc.sync.dma_start(out=fc0bb_sb[:], in_=fc0bb_in[:])
        fc1b_sb = const.tile([P, P], F32)
        nc.sync.dma_start(out=fc1b_sb[:], in_=fc1b_in[:])

        pq = psp.tile([G, P], F32, space="PSUM", tag="mm")
        for k in range(6):
            nc.tensor.matmul(
                out=pq[:],
                lhsT=qe_sb[:, k * G : (k + 1) * G],
                rhs=fc0w_sb[:, k * P : (k + 1) * P],
                start=(k == 0),
                stop=(k == 5),
            )
        qtmp = epi.tile([G, P], F32, tag="qtmp")
        nc.vector.tensor_tensor(
            out=qtmp[:], in0=pq[:], in1=fc0bb_sb[:G, :], op=mybir.AluOpType.add
        )
        qrelu = epi.tile([G, P], F32, tag="qrelu")
        nc.scalar.activation(
            out=qrelu[:], in_=qtmp[:], func=mybir.ActivationFunctionType.Relu
        )
        pqt = psh.tile([P, G], F32, space="PSUM", tag="hd")
        nc.tensor.transpose(out=pqt[:], in_=qrelu[:], identity=ident[:G, :G])
        qT = epi.tile([P, G], F32, tag="qT")
        nc.scalar.copy(out=qT[:], in_=pqt[:])
        pqq = psp.tile([G, P], F32, space="PSUM", tag="mm")
        nc.tensor.matmul(
            out=pqq[:], lhsT=qT[:], rhs=fc1b_sb[:], start=True, stop=True
        )
        qq_sb = const.tile([G, P], FDT, tag="qq_sb")
        nc.vector.tensor_copy(out=qq_sb[:], in_=pqq[:])

        # resident own-slice h~ buffers (self-loop term source), layer parity
        hs_keep = [
            const.tile([P, SLOT_ROWS], FDT, tag=f"hsk{i}", name=f"hsk{i}")
            for i in range(2)
        ]

        def allgather_block(l, j):
            r0 = j * SPB * P
            r1 = (j + 1) * SPB * P
            nc.gpsimd.collective_compute(
                "AllGather",
                mybir.AluOpType.bypass,
                replica_groups=[list(range(NCORES))],
                ins=[cc_in[r0:r1].opt()],
                outs=[tables[l][j * NCORES * SPB * P : (j + 1) * NCORES * SPB * P].opt()],
            )

        # ---- layer-0 table: replicated production  table0 = dinv*(x @ W0)
        XB = 8  # tiles per x stream block
        for i0 in range(0, NPOS, XB):
            ilim = min(i0 + XB, NPOS)
            xb = xp.tile([P, XB * P], FDT, tag="xb")
            nc.sync.dma_start(
                out=xb[:, : (ilim - i0) * P], in_=xT_perm[:, i0 * P : ilim * P]
            )
            for i in range(i0, ilim):
                pp = psp.tile([P, P], F32, space="PSUM", tag="mm")
                nc.tensor.matmul(
                    out=pp[:],
                    lhsT=xb[:, (i - i0) * P : (i - i0 + 1) * P],
                    rhs=W_sb[0][:],
                    start=True,
                    stop=True,
                )
                ht0 = l0p.tile([P, P], FDT, tag="ht0")
                nc.scalar.activation(
                    out=ht0[:],
                    in_=pp[:],
                    func=mybir.ActivationFunctionType.Copy,
                    scale=dinv_perm_sb[:, i : i + 1],
                )
                nc.sync.dma_start(out=table0[i * P : (i + 1) * P, :], in_=ht0[:])
        # own-slot h~0 for the self-loop terms
        for s0 in range(0, TPC, XB):
            slim = min(s0 + XB, TPC)
            xb = xp.tile([P, XB * P], FDT, tag="xb")
            nc.sync.dma_start(
                out=xb[:, : (slim - s0) * P], in_=xT_own[:, s0 * P : slim * P]
            )
            for s in range(s0, slim):
                pp = psp.tile([P, P], F32, space="PSUM", tag="mm")
                nc.tensor.matmul(
                    out=pp[:],
                    lhsT=xb[:, (s - s0) * P : (s - s0 + 1) * P],
                    rhs=W_sb[0][:],
                    start=True,
                    stop=True,
                )
                nc.scalar.activation(
                    out=hs_keep[0][:, s * P : (s + 1) * P],
                    in_=pp[:],
                    func=mybir.ActivationFunctionType.Copy,
                    scale=dinv_sb[:, s : s + 1],
                )

        # one large gather per (group, half), round-robin over 4 queues
        qctr = [0]

        def gather(dst_tile, src_ap, col_base, nchunks):
            nc.gpsimd.dma_gather(
                out_ap=dst_tile[:, : nchunks * P].rearrange(
                    "p (k q) -> p k q", q=P
                ),
                in_ap=src_ap,
                idxs_ap=idx_sb[:, col_base * 8 : (col_base + nchunks) * 8],
                num_idxs=nchunks * P,
                num_idxs_reg=nchunks * P,
                elem_size=P,
                single_packet=(nchunks <= 8),
                queue_num=qctr[0] % 4,
            )
            qctr[0] += 1

        NG = len(grp_info)
        PF = 2  # gather prefetch depth (groups)

        # ---- 3 aggregation layers
        for l in range(3):
            table = tables[l]
            gtiles = {}

            def issue_gather(gi):
                gs, col0, lo_tot, hi_tot = grp_info[gi]
                glo = gp.tile([P, LOMAX * P], FDT, tag="glo")
                if lo_tot:
                    gather(glo, table[0:HALF], col0, lo_tot)
                ghi = ghp.tile([P, HIMAX * P], FDT, tag="ghi")
                if hi_tot:
                    gather(ghi, table[HALF:NT], col0 + lo_tot, hi_tot)
                gtiles[gi] = (glo, ghi)

            for gi in range(min(PF, NG)):
                issue_gather(gi)

            for gi in range(NG):
                if gi + PF < NG:
                    issue_gather(gi + PF)
                gs, col0, lo_tot, hi_tot = grp_info[gi]
                glo, ghi = gtiles.pop(gi)
                ncols = len(gs) * P
                hT = htp.tile([P, GROUP * P], FDT, tag="hT")
                psg = psagg.tile([P, GROUP * P], F32, space="PSUM", tag="agg")
                for si, s in enumerate(gs):
                    nlo = int(nch[s, 0])
                    nhi = int(nch[s, 1])
                    ntot = nlo + nhi
                    lo_rel = int(slot_base[s, 0]) - col0  # within glo
                    hi_rel = int(slot_base[s, 1]) - col0 - lo_tot  # within ghi
                    db = int(dcol[s, 0])
                    # one-hot for this slot's chunks (lo then hi)
                    oh = ohp.tile([P, NTOTMAX * P], FDT, tag="oh")
                    if ntot:
                        nc.vector.tensor_tensor(
                            out=oh[:, : ntot * P].rearrange("p (k q) -> p k q", q=P),
                            in0=dstin_sb[:, db : db + ntot].to_broadcast(
                                [P, ntot, P]
                            ),
                            in1=iota_sb[:, : ntot * P].rearrange(
                                "p (k q) -> p k q", q=P
                            ),
                            op=mybir.AluOpType.is_equal,
                        )
                    # transposed aggregation: psT[feat, dst]
                    ps = psg[:, si * P : (si + 1) * P]
                    for k in range(nlo):
                        nc.tensor.matmul(
                            out=ps,
                            lhsT=glo[:, (lo_rel + k) * P : (lo_rel + k + 1) * P],
                            rhs=oh[:, k * P : (k + 1) * P],
                            start=(k == 0),
                            stop=False,
                        )
                    for k in range(nhi):
                        nc.tensor.matmul(
                            out=ps,
                            lhsT=ghi[:, (hi_rel + k) * P : (hi_rel + k + 1) * P],
                            rhs=oh[:, (nlo + k) * P : (nlo + k + 1) * P],
                            start=False,
                            stop=False,
                        )
                    # self-loop: psT += h~_slot^T
                    nc.tensor.matmul(
                        out=ps,
                        lhsT=hs_keep[l % 2][:, s * P : (s + 1) * P],
                        rhs=ident_r[:],
                        start=(ntot == 0),
                        stop=True,
                    )
                    # epilogue: hT = relu(dinv[dst] * psT + b_l)
                    tmp = epi.tile([P, P], F32, tag="tmp")
                    nc.vector.tensor_tensor(
                        out=tmp[:],
                        in0=ps,
                        in1=dinvB_sb[:, s * P : (s + 1) * P],
                        op=mybir.AluOpType.mult,
                    )
                    nc.scalar.activation(
                        out=hT[:, si * P : (si + 1) * P],
                        in_=tmp[:],
                        func=mybir.ActivationFunctionType.Relu,
                        bias=brow_sb[:, l : l + 1],
                    )
                    if l < 2:
                        # next-layer production: h~ = dinv * (h @ W)
                        pp2 = psp.tile([P, P], F32, space="PSUM", tag="mm")
                        nc.tensor.matmul(
                            out=pp2[:],
                            lhsT=hT[:, si * P : (si + 1) * P],
                            rhs=W_sb[l + 1][:],
                            start=True,
                            stop=True,
                        )
                        hs2 = hs_keep[(l + 1) % 2][:, s * P : (s + 1) * P]
                        nc.scalar.activation(
                            out=hs2,
                            in_=pp2[:],
                            func=mybir.ActivationFunctionType.Copy,
                            scale=dinv_sb[:, s : s + 1],
                        )
                        nc.sync.dma_start(
                            out=cc_in[s * P : (s + 1) * P, :], in_=hs2
                        )
                        if (s + 1) % SPB == 0:
                            allgather_block(l + 1, s // SPB)
                if l == 2:
                    # MLP head (transposed): uT = fc1a^T h3T + qq^T sel
                    s0 = gs[0]
                    pu = psh.tile([P, GROUP * P], F32, space="PSUM", tag="hd")
                    nc.tensor.matmul(
                        out=pu[:, :ncols],
                        lhsT=fc1a_sb[:],
                        rhs=hT[:, :ncols],
                        start=True,
                        stop=False,
                    )
                    nc.tensor.matmul(
                        out=pu[:, :ncols],
                        lhsT=qq_sb[:],
                        rhs=sel_sb[:, s0 * P : s0 * P + ncols],
                        start=False,
                        stop=True,
                    )
                    ur = epi.tile([P, GROUP * P], FDT, tag="ur")
                    nc.scalar.activation(
                        out=ur[:, :ncols],
                        in_=pu[:, :ncols],
                        func=mybir.ActivationFunctionType.Relu,
                        bias=fc1bcol_sb[:],
                    )
                    for si, s in enumerate(gs):
                        po = psp.tile([P, OUTC], F32, space="PSUM", tag="mm")
                        nc.tensor.matmul(
                            out=po[:],
                            lhsT=ur[:, si * P : (si + 1) * P],
                            rhs=fc2w_sb[:],
                            start=True,
                            stop=True,
                        )
                        ob = epi.tile([P, OUTC], F32, tag="ob")
                        nc.vector.tensor_tensor(
                            out=ob[:],
                            in0=po[:],
                            in1=fc2bb_sb[:],
                            op=mybir.AluOpType.add,
                        )
                        nc.sync.dma_start(
                            out=out_d[s * P : (s + 1) * P, :], in_=ob[:]
                        )
    nc.compile()
    return nc


# ---------------------------------------------------------------- interface
_CACHE = {}


def kernel(**inputs):
    trace = bool(int(os.environ.get("GCN_TRACE", "0")))
    if trace:
        _install_axon_prof()
    from concourse.bass_utils import run_bass_kernel_spmd

    x = np.ascontiguousarray(np.asarray(inputs["x"], dtype=np.float32))
    qe = np.asarray(inputs["question_embedding"], dtype=np.float32)
    pp = preprocess(inputs["edge_index"], inputs["batch"])
    nch = pp["nch"]

    key = tuple(nch.flatten().tolist())
    if key not in _CACHE:
        _CACHE[key] = build_program(
            nch, pp["dcol"], pp["grp_info"], pp["slot_base"], pp["TCH"]
        )
    nc = _CACHE[key]

    NTOTMAX = int((nch[:, 0] + nch[:, 1]).max())

    W = [np.asarray(inputs[f"W{i}"], np.float32) for i in range(3)]
    b = [np.asarray(inputs[f"b{i}"], np.float32) for i in range(3)]
    fc0_w = np.asarray(inputs["fc0_w"], np.float32)
    fc0_b = np.asarray(inputs["fc0_b"], np.float32)
    fc1_w = np.asarray(inputs["fc1_w"], np.float32)
    fc1_b = np.asarray(inputs["fc1_b"], np.float32)
    fc2_w = np.asarray(inputs["fc2_w"], np.float32)
    fc2_b = np.asarray(inputs["fc2_b"], np.float32)

    # x permuted to table order (replicated layer-0 production input)
    xT_perm = np.zeros((P, NT), dtype=BF16)
    valid = pp["pos_node"] >= 0
    xT_perm[:, valid] = x[pp["pos_node"][valid]].T.astype(BF16)

    iota = np.broadcast_to(np.arange(P, dtype=np.float32), (P, P))
    iota_rep = np.ascontiguousarray(np.tile(iota, (1, NTOTMAX)).astype(BF16))
    common = {
        "xT_perm": xT_perm,
        "iota_in": iota_rep,
        "W0": W[0].astype(BF16),
        "W1": W[1].astype(BF16),
        "W2": W[2].astype(BF16),
        "brow": np.stack(b, axis=1).astype(np.float32).copy(),
        "dinv_perm": pp["dinv_perm"],
        "qeT": np.ascontiguousarray(qe.T),
        "fc0w": fc0_w,
        "fc0bb": np.broadcast_to(fc0_b, (P, P)).copy(),
        "fc1a": np.ascontiguousarray(fc1_w[:P]).astype(BF16),
        "fc1b": np.ascontiguousarray(fc1_w[P:]),
        "fc1bcol": fc1_b.reshape(P, 1).copy(),
        "fc2w": fc2_w.astype(BF16),
        "fc2bb": np.broadcast_to(fc2_b, (P, OUTC)).copy(),
    }

    in_maps = []
    for c in range(NCORES):
        xTc = np.zeros((P, SLOT_ROWS), dtype=BF16)
        validc = pp["node_perm"][c] >= 0
        xTc[:, validc] = x[pp["node_perm"][c][validc]].T.astype(BF16)
        m = dict(common)
        m["xT_own"] = xTc
        m["idx_in"] = np.ascontiguousarray(pp["idx_T"][c])
        m["dstin"] = np.ascontiguousarray(pp["dstin_T"][c].astype(BF16))
        m["dinv_in"] = np.ascontiguousarray(pp["dinv_slot"][c])
        m["dinvB_in"] = np.ascontiguousarray(pp["dinvB"][c])
        m["sel_in"] = np.ascontiguousarray(pp["sel"][c].astype(BF16))
        in_maps.append(m)

    res = run_bass_kernel_spmd(
        nc,
        in_maps,
        list(range(NCORES)),
        trace=trace,
    )
    kernel.last_result = res

    out = np.zeros((N, OUTC), dtype=np.float32)
    for c in range(NCORES):
        validc = pp["node_perm"][c] >= 0
        out[pp["node_perm"][c][validc]] = res.results[c]["out"][validc]
    return out


# revision 8
# speedup vs baseline: 1.2337x; 1.2337x over previous
"""Trainium2 Bass kernel for the GCN model (nn_GCNModel_57853209477141).

Model: 3x GCNConv(128->128, sym-norm with self loops) with ReLU, question
embedding MLP, concat, 2-layer MLP head -> [50000, 32].

Strategy (8 NeuronCores, single SPMD launch):
- dst-node sharding: global tiles of 128 nodes; snake-dealt across cores by
  edge count so one compile-time chunk schedule serves all 8 cores.
- GCN norm factorization: agg[v] = dinv[v] * sum_{e->v} (dinv*h)[src_e]; the
  per-edge norm disappears by storing h~ = dinv*h in the gather table.
- layer-0 table is computed REPLICATED on every core (x is replicated), so
  no AllGather is needed for it; layers 1/2 tables are AllGathered per
  7-slot block from the production epilogues (overlapped with compute).
- TRANSPOSED aggregation: psT[feat, dst] += glo_chunk.T @ onehot_chunk, so
  the epilogue produces h^T directly (no PE transposes anywhere), the layer
  bias is a native per-partition activation bias, and dinv[dst] is applied
  with one DVE multiply against a host-precomputed broadcast plane.
- gathers: ONE large bf16 dma_gather per (4-slot group x table-half)
  (int16 gather indices address at most 32768 rows -> table split at row
  32768), prefetched 2 groups ahead, round-robin over 4 SWDGE queues.
- question path: qq = relu(qe@fc0+b)@fc1[128:] kept on-chip; added in the
  head via a one-hot-over-graphs matmul (sel plane precomputed on host).
- head: u^T = fc1a.T @ h3^T + qq.T @ sel; out = relu(u)^T @ fc2 + fc2_b.

Host preprocessing is index work only (sharding, edge sort, index planes);
all O(E*F) / O(N*F*F) float work runs on device.
"""
import os
import sys
import types
from contextlib import ExitStack

import numpy as np

# ---------------------------------------------------------------- constants
N = 50000
E = 800000
G = 64
P = 128
NCORES = 8
TPC = 49  # tile slots per core
SPB = 7  # slots per AllGather block
NBLK = 7
SLOT_ROWS = TPC * P  # 6272
NT = NCORES * SLOT_ROWS  # 50176
NPOS = NT // P  # 392 global table tile positions
HALF = 32768  # int16 gather index limit -> table split row
QD = 768
OUTC = 32
GROUP = 4  # slots per gather group / head group

BF16 = np.dtype("bfloat16")


def _install_axon_prof():
    """Register NTFF profile hook if the image's antenv lacks it; neuter
    bucket upload (zero-egress). Harmless when running without tracing."""
    try:
        from antenv import axon_hooks  # noqa: F401
    except ImportError:
        try:
            import antenv
            from trn_agent_boot.trn_boot import _ntff_profile_via_ctypes

            hook = _ntff_profile_via_ctypes("/opt/axon/libaxon_pjrt.so")
            mod = types.ModuleType("antenv.axon_hooks")
            mod.get_axon_ntff_profile_hook = lambda: hook
            mod.set_axon_ntff_profile_hook = lambda h: None
            sys.modules["antenv.axon_hooks"] = mod
            antenv.axon_hooks = mod
        except Exception:
            pass
    try:
        import concourse.bass_utils as bu

        bu.upload_artifacts = lambda tmpdir: "local://" + str(tmpdir)
    except Exception:
        pass


def _wrap16(arr):
    """int array -> [128, len/16] int16 plane (idx i at partition i%16,
    col i//16; replicated to all 8 gpsimd core groups)."""
    m = np.asarray(arr, dtype=np.int16).reshape(-1, 16).T
    return np.tile(m, (8, 1))


# ---------------------------------------------------------------- host prep
def preprocess(edge_index, batch):
    src = np.asarray(edge_index[0], dtype=np.int64)
    dst = np.asarray(edge_index[1], dtype=np.int64)
    deg = (np.bincount(dst, minlength=N) + 1).astype(np.float64)
    dinv = (1.0 / np.sqrt(deg)).astype(np.float32)

    n_tiles = (N + P - 1) // P  # 391
    tile_of_node = np.arange(N) // P
    dst_tile = dst // P
    tile_counts = np.bincount(dst_tile, minlength=n_tiles)

    # snake-deal tiles (sorted by edge count desc) across cores
    order_all = np.argsort(-tile_counts, kind="stable")
    core_tiles = [[] for _ in range(NCORES)]
    for r in range(TPC):
        batch_t = order_all[r * NCORES : (r + 1) * NCORES]
        seq = range(NCORES) if r % 2 == 0 else range(NCORES - 1, -1, -1)
        for j, c in enumerate(seq):
            core_tiles[c].append(int(batch_t[j]) if j < len(batch_t) else -1)

    core_of_tile = np.full(n_tiles, -1, dtype=np.int64)
    slot_of_tile = np.full(n_tiles, -1, dtype=np.int64)
    for c in range(NCORES):
        for s, t in enumerate(core_tiles[c]):
            if t >= 0:
                core_of_tile[t] = c
                slot_of_tile[t] = s

    # block-major table row for every node (same layout for all 3 layers)
    blk = slot_of_tile[tile_of_node] // SPB
    table_row = (
        blk * (NCORES * SPB * P)
        + core_of_tile[tile_of_node] * (SPB * P)
        + (slot_of_tile[tile_of_node] % SPB) * P
        + (np.arange(N) % P)
    )

    order = np.argsort(dst_tile, kind="stable")
    src_sorted = src[order]
    dst_sorted = dst[order]
    sorted_tiles = dst_tile[order]
    tile_starts = np.searchsorted(sorted_tiles, np.arange(n_tiles))
    tile_ends = np.searchsorted(sorted_tiles, np.arange(n_tiles), side="right")

    src_rows = table_row[src_sorted]
    is_lo = src_rows < HALF

    # per-(core, slot, half) edge lists + common chunk schedule
    cnt = np.zeros((NCORES, TPC, 2), dtype=np.int64)
    elists = [[None] * TPC for _ in range(NCORES)]  # (rows_lo, din_lo, rows_hi, din_hi)
    for c in range(NCORES):
        for s in range(TPC):
            t = core_tiles[c][s]
            if t < 0:
                elists[c][s] = (
                    np.zeros(0, np.int64), np.zeros(0, np.int64),
                    np.zeros(0, np.int64), np.zeros(0, np.int64),
                )
                continue
            lo_, hi_ = tile_starts[t], tile_ends[t]
            rows = src_rows[lo_:hi_]
            din = dst_sorted[lo_:hi_] % P
            m = is_lo[lo_:hi_]
            elists[c][s] = (rows[m], din[m], rows[~m] - HALF, din[~m])
            cnt[c, s, 0] = int(m.sum())
            cnt[c, s, 1] = int((~m).sum())

    nch = np.ceil(cnt.max(axis=0) / P).astype(np.int64)  # [TPC, 2]

    # gather-call schedule: chunk columns [grp lo (slot-major) | grp hi]
    groups = [list(range(g, min(g + GROUP, TPC))) for g in range(0, TPC, GROUP)]
    slot_base = np.zeros((TPC, 2), dtype=np.int64)  # gather-col base per slot
    grp_info = []  # (slots, col0, lo_tot, hi_tot)
    cur = 0
    for gs in groups:
        col0 = cur
        for s in gs:
            slot_base[s, 0] = cur
            cur += nch[s, 0]
        for s in gs:
            slot_base[s, 1] = cur
            cur += nch[s, 1]
        lo_tot = int(nch[gs, 0].sum())
        hi_tot = int(nch[gs, 1].sum())
        grp_info.append((gs, col0, lo_tot, hi_tot))
    TCH = int(cur)

    # dstin plane is SLOT-major: [slot: lo chunks | hi chunks] so one DVE
    # is_equal per slot builds the whole one-hot for that slot.
    dcol = np.zeros((TPC, 2), dtype=np.int64)
    cur = 0
    for s in range(TPC):
        dcol[s, 0] = cur
        cur += nch[s, 0]
        dcol[s, 1] = cur
        cur += nch[s, 1]
    assert cur == TCH

    idx_T = np.zeros((NCORES, 128, TCH * 8), dtype=np.int16)
    dstin_T = np.full((NCORES, 128, TCH), -1.0, dtype=np.float32)
    for c in range(NCORES):
        for s in range(TPC):
            rows_lo, din_lo, rows_hi, din_hi = elists[c][s]
            for h, (rows, din) in enumerate(((rows_lo, din_lo), (rows_hi, din_hi))):
                nchunks = int(nch[s, h])
                if nchunks == 0:
                    continue
                pad = nchunks * P
                rbuf = np.zeros(pad, dtype=np.int64)
                rbuf[: len(rows)] = rows
                dbuf = np.full(pad, -1.0, dtype=np.float32)
                dbuf[: len(din)] = din.astype(np.float32)
                b = int(slot_base[s, h])
                idx_T[c, :, b * 8 : (b + nchunks) * 8] = _wrap16(rbuf)
                db = int(dcol[s, h])
                dstin_T[c, :, db : db + nchunks] = dbuf.reshape(nchunks, P).T
    del elists

    # per-core planes: dinv per slot (per-partition), dinv broadcast across
    # partitions (free-dim scale in transposed epilogue), graph-select
    # one-hot, node permutation
    dinv_slot = np.zeros((NCORES, P, TPC), dtype=np.float32)
    dinvB = np.zeros((NCORES, P, SLOT_ROWS), dtype=np.float32)
    sel = np.zeros((NCORES, G, SLOT_ROWS), dtype=np.float32)
    node_perm = np.full((NCORES, SLOT_ROWS), -1, dtype=np.int64)
    batch = np.asarray(batch, dtype=np.int64)
    for c in range(NCORES):
        for s in range(TPC):
            t = core_tiles[c][s]
            if t < 0:
                continue
            v0 = t * P
            v1 = min(v0 + P, N)
            n = v1 - v0
            dinv_slot[c, :n, s] = dinv[v0:v1]
            dinvB[c, :, s * P : s * P + n] = dinv[v0:v1][None, :]
            sel[c, batch[v0:v1], s * P + np.arange(n)] = 1.0
            node_perm[c, s * P : s * P + n] = np.arange(v0, v1)

    # replicated layer-0 production: x columns permuted to table order
    pos_node = np.full(NT, -1, dtype=np.int64)
    pos_node[table_row] = np.arange(N)
    dinv_perm = np.zeros((P, NPOS), dtype=np.float32)
    valid = pos_node >= 0
    dp = np.zeros(NT, dtype=np.float32)
    dp[valid] = dinv[pos_node[valid]]
    dinv_perm[:, :] = dp.reshape(NPOS, P).T

    return dict(
        nch=nch,
        dcol=dcol,
        grp_info=grp_info,
        slot_base=slot_base,
        TCH=TCH,
        idx_T=idx_T,
        dstin_T=dstin_T,
        dinv_slot=dinv_slot,
        dinvB=dinvB,
        sel=sel,
        node_perm=node_perm,
        pos_node=pos_node,
        dinv_perm=dinv_perm,
    )


# ------------------------------------------------------------- bass program
def build_program(nch, dcol, grp_info, slot_base, TCH):
    import concourse.bacc as bacc
    import concourse.bass as bass
    import concourse.tile as tile
    from concourse import library_config, mybir
    from concourse.masks import make_identity

    FDT = mybir.dt.bfloat16
    F32 = mybir.dt.float32
    I16 = mybir.dt.int16

    LOMAX = max(lo for _, _, lo, _ in grp_info)
    HIMAX = max(hi for _, _, _, hi in grp_info)
    NTOTMAX = int((nch[:, 0] + nch[:, 1]).max())

    nc = bacc.Bacc("TRN2", target_bir_lowering=False, num_swdge_queues=4)
    dp = nc.declare_dram_parameter
    xT_perm = dp("xT_perm", [P, NT], FDT, isOutput=False)  # replicated, table order
    xT_own = dp("xT_own", [P, SLOT_ROWS], FDT, isOutput=False)  # own slots
    idx_in = dp("idx_in", [P, TCH * 8], I16, isOutput=False)
    dstin = dp("dstin", [P, TCH], FDT, isOutput=False)
    iota_in = dp("iota_in", [P, NTOTMAX * P], FDT, isOutput=False)
    dinv_in = dp("dinv_in", [P, TPC], F32, isOutput=False)
    dinvB_in = dp("dinvB_in", [P, SLOT_ROWS], F32, isOutput=False)
    dinv_perm_in = dp("dinv_perm", [P, NPOS], F32, isOutput=False)
    sel_in = dp("sel_in", [G, SLOT_ROWS], FDT, isOutput=False)
    W_in = [dp(f"W{i}", [P, P], FDT, isOutput=False) for i in range(3)]
    brow_in = dp("brow", [P, 3], F32, isOutput=False)
    qeT_in = dp("qeT", [QD, G], F32, isOutput=False)
    fc0w_in = dp("fc0w", [QD, P], F32, isOutput=False)
    fc0bb_in = dp("fc0bb", [P, P], F32, isOutput=False)
    fc1a_in = dp("fc1a", [P, P], FDT, isOutput=False)
    fc1b_in = dp("fc1b", [P, P], F32, isOutput=False)
    fc1bcol_in = dp("fc1bcol", [P, 1], F32, isOutput=False)
    fc2w_in = dp("fc2w", [P, OUTC], FDT, isOutput=False)
    fc2bb_in = dp("fc2bb", [P, OUTC], F32, isOutput=False)
    out_d = dp("out", [SLOT_ROWS, OUTC], F32, isOutput=True)

    cc_in = nc.dram_tensor("cc_in", [SLOT_ROWS, P], FDT)
    table0 = nc.dram_tensor("table0", [NT, P], FDT)
    tables = [
        table0,
        nc.dram_tensor("table1", [NT, P], FDT, addr_space="Shared"),
        nc.dram_tensor("table2", [NT, P], FDT, addr_space="Shared"),
    ]

    with tile.TileContext(nc) as tc, ExitStack() as ctx:
        nc.gpsimd.load_library(library_config.mlp)

        const = ctx.enter_context(tc.tile_pool(name="const", bufs=1))
        xp = ctx.enter_context(tc.tile_pool(name="xp", bufs=3))
        l0p = ctx.enter_context(tc.tile_pool(name="l0p", bufs=4))
        gp = ctx.enter_context(tc.tile_pool(name="gp", bufs=3))
        ghp = ctx.enter_context(tc.tile_pool(name="ghp", bufs=3))
        ohp = ctx.enter_context(tc.tile_pool(name="ohp", bufs=4))
        htp = ctx.enter_context(tc.tile_pool(name="htp", bufs=3))
        psagg = ctx.enter_context(tc.tile_pool(name="psagg", bufs=2, space="PSUM"))
        psp = ctx.enter_context(tc.tile_pool(name="psp", bufs=3, space="PSUM"))
        psh = ctx.enter_context(tc.tile_pool(name="psh", bufs=2, space="PSUM"))
        epi = ctx.enter_context(tc.tile_pool(name="epi", bufs=4))

        # ---- constants
        idx_sb = const.tile([P, TCH * 8], I16)
        nc.scalar.dma_start(out=idx_sb[:], in_=idx_in[:])
        dstin_sb = const.tile([P, TCH], FDT)
        nc.scalar.dma_start(out=dstin_sb[:], in_=dstin[:])
        iota_sb = const.tile([P, NTOTMAX * P], FDT)
        nc.scalar.dma_start(out=iota_sb[:], in_=iota_in[:])
        dinv_sb = const.tile([P, TPC], F32)
        nc.sync.dma_start(out=dinv_sb[:], in_=dinv_in[:])
        dinvB_sb = const.tile([P, SLOT_ROWS], F32)
        nc.sync.dma_start(out=dinvB_sb[:], in_=dinvB_in[:])
        dinv_perm_sb = const.tile([P, NPOS], F32)
        nc.sync.dma_start(out=dinv_perm_sb[:], in_=dinv_perm_in[:])
        sel_sb = const.tile([G, SLOT_ROWS], FDT)
        nc.sync.dma_start(out=sel_sb[:], in_=sel_in[:])
        W_sb = []
        for i in range(3):
            w = const.tile([P, P], FDT, tag=f"W{i}")
            nc.sync.dma_start(out=w[:], in_=W_in[i][:])
            W_sb.append(w)
        brow_sb = const.tile([P, 3], F32)
        nc.sync.dma_start(out=brow_sb[:], in_=brow_in[:])
        fc1a_sb = const.tile([P, P], FDT)
        nc.sync.dma_start(out=fc1a_sb[:], in_=fc1a_in[:])
        fc1bcol_sb = const.tile([P, 1], F32)
        nc.sync.dma_start(out=fc1bcol_sb[:], in_=fc1bcol_in[:])
        fc2w_sb = const.tile([P, OUTC], FDT)
        nc.sync.dma_start(out=fc2w_sb[:], in_=fc2w_in[:])
        fc2bb_sb = const.tile([P, OUTC], F32)
        nc.sync.dma_start(out=fc2bb_sb[:], in_=fc2bb_in[:])
        ident = const.tile([P, P], F32)
        make_identity(nc, ident[:])
        ident_r = const.tile([P, P], FDT, tag="ident_r")
        nc.vector.tensor_copy(out=ident_r[:], in_=ident[:])

        # ---- question path: qq = relu(qe@fc0+fc0_b)@fc1b  (bf16, on-chip;
        # fc1_b enters later as the head activation bias)
        qe_sb = const.tile([P, 6 * G], F32)
        fc0w_sb = const.tile([P, 6 * P], F32)
        for k in range(6):
            nc.sync.dma_start(
                out=qe_sb[:, k * G : (k + 1) * G], in_=qeT_in[k * P : (k + 1) * P, :]
            )
            nc.sync.dma_start(
                out=fc0w_sb[:, k * P : (k + 1) * P],
                in_=fc0w_in[k * P : (k + 1) * P, :],
            )
        fc0bb_sb = const.tile([P, P], F32)
        nc.sync.dma_start(out=fc0bb_sb[:], in_=fc0bb_in[:])
        fc1b_sb = const.tile([P, P], F32)
        nc.sync.dma_start(out=fc1b_sb[:], in_=fc1b_in[:])

        pq = psp.tile([G, P], F32, space="PSUM", tag="mm")
        for k in range(6):
            nc.tensor.matmul(
                out=pq[:],
                lhsT=qe_sb[:, k * G : (k + 1) * G],
                rhs=fc0w_sb[:, k * P : (k + 1) * P],
                start=(k == 0),
                stop=(k == 5),
            )
        qtmp = epi.tile([G, P], F32, tag="qtmp")
        nc.vector.tensor_tensor(
            out=qtmp[:], in0=pq[:], in1=fc0bb_sb[:G, :], op=mybir.AluOpType.add
        )
        qrelu = epi.tile([G, P], F32, tag="qrelu")
        nc.scalar.activation(
            out=qrelu[:], in_=qtmp[:], func=mybir.ActivationFunctionType.Relu
        )
        pqt = psh.tile([P, G], F32, space="PSUM", tag="hd")
        nc.tensor.transpose(out=pqt[:], in_=qrelu[:], identity=ident[:G, :G])
        qT = epi.tile([P, G], F32, tag="qT")
        nc.scalar.copy(out=qT[:], in_=pqt[:])
        pqq = psp.tile([G, P], F32, space="PSUM", tag="mm")
        nc.tensor.matmul(
            out=pqq[:], lhsT=qT[:], rhs=fc1b_sb[:], start=True, stop=True
        )
        qq_sb = const.tile([G, P], FDT, tag="qq_sb")
        nc.vector.tensor_copy(out=qq_sb[:], in_=pqq[:])

        # resident own-slice h~ buffers (self-loop term source), layer parity
        hs_keep = [
            const.tile([P, SLOT_ROWS], FDT, tag=f"hsk{i}", name=f"hsk{i}")
            for i in range(2)
        ]

        def allgather_block(l, j):
            r0 = j * SPB * P
            r1 = (j + 1) * SPB * P
            nc.gpsimd.collective_compute(
                "AllGather",
                mybir.AluOpType.bypass,
                replica_groups=[list(range(NCORES))],
                ins=[cc_in[r0:r1].opt()],
                outs=[tables[l][j * NCORES * SPB * P : (j + 1) * NCORES * SPB * P].opt()],
            )

        # ---- layer-0 table: replicated production  table0 = dinv*(x @ W0)
        XB = 8  # tiles per x stream block
        for i0 in range(0, NPOS, XB):
            ilim = min(i0 + XB, NPOS)
            xb = xp.tile([P, XB * P], FDT, tag="xb")
            nc.sync.dma_start(
                out=xb[:, : (ilim - i0) * P], in_=xT_perm[:, i0 * P : ilim * P]
            )
            for i in range(i0, ilim):
                pp = psp.tile([P, P], F32, space="PSUM", tag="mm")
                nc.tensor.matmul(
                    out=pp[:],
                    lhsT=xb[:, (i - i0) * P : (i - i0 + 1) * P],
                    rhs=W_sb[0][:],
                    start=True,
                    stop=True,
                )
                ht0 = l0p.tile([P, P], FDT, tag="ht0")
                nc.scalar.activation(
                    out=ht0[:],
                    in_=pp[:],
                    func=mybir.ActivationFunctionType.Copy,
                    scale=dinv_perm_sb[:, i : i + 1],
                )
                nc.sync.dma_start(out=table0[i * P : (i + 1) * P, :], in_=ht0[:])
        # own-slot h~0 for the self-loop terms
        for s0 in range(0, TPC, XB):
            slim = min(s0 + XB, TPC)
            xb = xp.tile([P, XB * P], FDT, tag="xb")
            nc.sync.dma_start(
                out=xb[:, : (slim - s0) * P], in_=xT_own[:, s0 * P : slim * P]
            )
            for s in range(s0, slim):
                pp = psp.tile([P, P], F32, space="PSUM", tag="mm")
                nc.tensor.matmul(
                    out=pp[:],
                    lhsT=xb[:, (s - s0) * P : (s - s0 + 1) * P],
                    rhs=W_sb[0][:],
                    start=True,
                    stop=True,
                )
                nc.scalar.activation(
                    out=hs_keep[0][:, s * P : (s + 1) * P],
                    in_=pp[:],
                    func=mybir.ActivationFunctionType.Copy,
                    scale=dinv_sb[:, s : s + 1],
                )

        # multi-queue gather helper: <=8-chunk single-packet calls, round-robin
        # over the 4 SWDGE queues (queues generate descriptors in parallel)
        qctr = [0]

        def gather(dst_tile, src_ap, col_base, nchunks):
            k = 0
            while k < nchunks:
                nb = min(8, nchunks - k)
                nc.gpsimd.dma_gather(
                    out_ap=dst_tile[
                        :, k * P : (k + nb) * P
                    ].rearrange("p (k q) -> p k q", q=P),
                    in_ap=src_ap,
                    idxs_ap=idx_sb[:, (col_base + k) * 8 : (col_base + k + nb) * 8],
                    num_idxs=nb * P,
                    num_idxs_reg=nb * P,
                    elem_size=P,
                    single_packet=True,
                    queue_num=qctr[0] % 4,
                )
                qctr[0] += 1
                k += nb

        NG = len(grp_info)
        PF = 2  # gather prefetch depth (groups)

        # ---- 3 aggregation layers
        for l in range(3):
            table = tables[l]
            gtiles = {}

            def issue_gather(gi):
                gs, col0, lo_tot, hi_tot = grp_info[gi]
                glo = gp.tile([P, LOMAX * P], FDT, tag="glo")
                if lo_tot:
                    gather(glo, table[0:HALF], col0, lo_tot)
                ghi = ghp.tile([P, HIMAX * P], FDT, tag="ghi")
                if hi_tot:
                    gather(ghi, table[HALF:NT], col0 + lo_tot, hi_tot)
                gtiles[gi] = (glo, ghi)

            for gi in range(min(PF, NG)):
                issue_gather(gi)

            for gi in range(NG):
                if gi + PF < NG:
                    issue_gather(gi + PF)
                gs, col0, lo_tot, hi_tot = grp_info[gi]
                glo, ghi = gtiles.pop(gi)
                ncols = len(gs) * P
                hT = htp.tile([P, GROUP * P], FDT, tag="hT")
                psg = psagg.tile([P, GROUP * P], F32, space="PSUM", tag="agg")
                for si, s in enumerate(gs):
                    nlo = int(nch[s, 0])
                    nhi = int(nch[s, 1])
                    ntot = nlo + nhi
                    lo_rel = int(slot_base[s, 0]) - col0  # within glo
                    hi_rel = int(slot_base[s, 1]) - col0 - lo_tot  # within ghi
                    db = int(dcol[s, 0])
                    # one-hot for this slot's chunks (lo then hi)
                    oh = ohp.tile([P, NTOTMAX * P], FDT, tag="oh")
                    if ntot:
                        nc.vector.tensor_tensor(
                            out=oh[:, : ntot * P].rearrange("p (k q) -> p k q", q=P),
                            in0=dstin_sb[:, db : db + ntot].to_broadcast(
                                [P, ntot, P]
                            ),
                            in1=iota_sb[:, : ntot * P].rearrange(
                                "p (k q) -> p k q", q=P
                            ),
                            op=mybir.AluOpType.is_equal,
                        )
                    # transposed aggregation: psT[feat, dst]
                    ps = psg[:, si * P : (si + 1) * P]
                    for k in range(nlo):
                        nc.tensor.matmul(
                            out=ps,
                            lhsT=glo[:, (lo_rel + k) * P : (lo_rel + k + 1) * P],
                            rhs=oh[:, k * P : (k + 1) * P],
                            start=(k == 0),
                            stop=False,
                        )
                    for k in range(nhi):
                        nc.tensor.matmul(
                            out=ps,
                            lhsT=ghi[:, (hi_rel + k) * P : (hi_rel + k + 1) * P],
                            rhs=oh[:, (nlo + k) * P : (nlo + k + 1) * P],
                            start=False,
                            stop=False,
                        )
                    # self-loop: psT += h~_slot^T
                    nc.tensor.matmul(
                        out=ps,
                        lhsT=hs_keep[l % 2][:, s * P : (s + 1) * P],
                        rhs=ident_r[:],
                        start=(ntot == 0),
                        stop=True,
                    )
                    # epilogue: hT = relu(dinv[dst] * psT + b_l)
                    tmp = epi.tile([P, P], F32, tag="tmp")
                    nc.vector.tensor_tensor(
                        out=tmp[:],
                        in0=ps,
                        in1=dinvB_sb[:, s * P : (s + 1) * P],
                        op=mybir.AluOpType.mult,
                    )
                    nc.scalar.activation(
                        out=hT[:, si * P : (si + 1) * P],
                        in_=tmp[:],
                        func=mybir.ActivationFunctionType.Relu,
                        bias=brow_sb[:, l : l + 1],
                    )
                    if l < 2:
                        # next-layer production: h~ = dinv * (h @ W)
                        pp2 = psp.tile([P, P], F32, space="PSUM", tag="mm")
                        nc.tensor.matmul(
                            out=pp2[:],
                            lhsT=hT[:, si * P : (si + 1) * P],
                            rhs=W_sb[l + 1][:],
                            start=True,
                            stop=True,
                        )
                        hs2 = hs_keep[(l + 1) % 2][:, s * P : (s + 1) * P]
                        nc.scalar.activation(
                            out=hs2,
                            in_=pp2[:],
                            func=mybir.ActivationFunctionType.Copy,
                            scale=dinv_sb[:, s : s + 1],
                        )
                        nc.sync.dma_start(
                            out=cc_in[s * P : (s + 1) * P, :], in_=hs2
                        )
                        if (s + 1) % SPB == 0:
                            allgather_block(l + 1, s // SPB)
                if l == 2:
                    # MLP head (transposed): uT = fc1a^T h3T + qq^T sel
                    s0 = gs[0]
                    pu = psh.tile([P, GROUP * P], F32, space="PSUM", tag="hd")
                    nc.tensor.matmul(
                        out=pu[:, :ncols],
                        lhsT=fc1a_sb[:],
                        rhs=hT[:, :ncols],
                        start=True,
                        stop=False,
                    )
                    nc.tensor.matmul(
                        out=pu[:, :ncols],
                        lhsT=qq_sb[:],
                        rhs=sel_sb[:, s0 * P : s0 * P + ncols],
                        start=False,
                        stop=True,
                    )
                    ur = epi.tile([P, GROUP * P], FDT, tag="ur")
                    nc.scalar.activation(
                        out=ur[:, :ncols],
                        in_=pu[:, :ncols],
                        func=mybir.ActivationFunctionType.Relu,
                        bias=fc1bcol_sb[:],
                    )
                    for si, s in enumerate(gs):
                        po = psp.tile([P, OUTC], F32, space="PSUM", tag="mm")
                        nc.tensor.matmul(
                            out=po[:],
                            lhsT=ur[:, si * P : (si + 1) * P],
                            rhs=fc2w_sb[:],
                            start=True,
                            stop=True,
                        )
                        ob = epi.tile([P, OUTC], F32, tag="ob")
                        nc.vector.tensor_tensor(
                            out=ob[:],
                            in0=po[:],
                            in1=fc2bb_sb[:],
                            op=mybir.AluOpType.add,
                        )
                        nc.sync.dma_start(
                            out=out_d[s * P : (s + 1) * P, :], in_=ob[:]
                        )
    nc.compile()
    return nc


# ---------------------------------------------------------------- interface
_CACHE = {}


def kernel(**inputs):
    trace = bool(int(os.environ.get("GCN_TRACE", "0")))
    if trace:
        _install_axon_prof()
    from concourse.bass_utils import run_bass_kernel_spmd

    x = np.ascontiguousarray(np.asarray(inputs["x"], dtype=np.float32))
    qe = np.asarray(inputs["question_embedding"], dtype=np.float32)
    pp = preprocess(inputs["edge_index"], inputs["batch"])
    nch = pp["nch"]

    key = tuple(nch.flatten().tolist())
    if key not in _CACHE:
        _CACHE[key] = build_program(
            nch, pp["dcol"], pp["grp_info"], pp["slot_base"], pp["TCH"]
        )
    nc = _CACHE[key]

    NTOTMAX = int((nch[:, 0] + nch[:, 1]).max())

    W = [np.asarray(inputs[f"W{i}"], np.float32) for i in range(3)]
    b = [np.asarray(inputs[f"b{i}"], np.float32) for i in range(3)]
    fc0_w = np.asarray(inputs["fc0_w"], np.float32)
    fc0_b = np.asarray(inputs["fc0_b"], np.float32)
    fc1_w = np.asarray(inputs["fc1_w"], np.float32)
    fc1_b = np.asarray(inputs["fc1_b"], np.float32)
    fc2_w = np.asarray(inputs["fc2_w"], np.float32)
    fc2_b = np.asarray(inputs["fc2_b"], np.float32)

    # x permuted to table order (replicated layer-0 production input)
    xT_perm = np.zeros((P, NT), dtype=BF16)
    valid = pp["pos_node"] >= 0
    xT_perm[:, valid] = x[pp["pos_node"][valid]].T.astype(BF16)

    iota = np.broadcast_to(np.arange(P, dtype=np.float32), (P, P))
    iota_rep = np.ascontiguousarray(np.tile(iota, (1, NTOTMAX)).astype(BF16))
    common = {
        "xT_perm": xT_perm,
        "iota_in": iota_rep,
        "W0": W[0].astype(BF16),
        "W1": W[1].astype(BF16),
        "W2": W[2].astype(BF16),
        "brow": np.stack(b, axis=1).astype(np.float32).copy(),
        "dinv_perm": pp["dinv_perm"],
        "qeT": np.ascontiguousarray(qe.T),
        "fc0w": fc0_w,
        "fc0bb": np.broadcast_to(fc0_b, (P, P)).copy(),
        "fc1a": np.ascontiguousarray(fc1_w[:P]).astype(BF16),
        "fc1b": np.ascontiguousarray(fc1_w[P:]),
        "fc1bcol": fc1_b.reshape(P, 1).copy(),
        "fc2w": fc2_w.astype(BF16),
        "fc2bb": np.broadcast_to(fc2_b, (P, OUTC)).copy(),
    }

    in_maps = []
    for c in range(NCORES):
        xTc = np.zeros((P, SLOT_ROWS), dtype=BF16)
        validc = pp["node_perm"][c] >= 0
        xTc[:, validc] = x[pp["node_perm"][c][validc]].T.astype(BF16)
        m = dict(common)
        m["xT_own"] = xTc
        m["idx_in"] = np.ascontiguousarray(pp["idx_T"][c])
        m["dstin"] = np.ascontiguousarray(pp["dstin_T"][c].astype(BF16))
        m["dinv_in"] = np.ascontiguousarray(pp["dinv_slot"][c])
        m["dinvB_in"] = np.ascontiguousarray(pp["dinvB"][c])
        m["sel_in"] = np.ascontiguousarray(pp["sel"][c].astype(BF16))
        in_maps.append(m)

    res = run_bass_kernel_spmd(
        nc,
        in_maps,
        list(range(NCORES)),
        trace=trace,
    )
    kernel.last_result = res

    out = np.zeros((N, OUTC), dtype=np.float32)
    for c in range(NCORES):
        validc = pp["node_perm"][c] >= 0
        out[pp["node_perm"][c][validc]] = res.results[c]["out"][validc]
    return out


# revision 10
# speedup vs baseline: 1.6313x; 1.3222x over previous
"""Trainium2 Bass kernel for the GCN model (nn_GCNModel_57853209477141).

Model: 3x GCNConv(128->128, sym-norm with self loops) with ReLU, question
embedding MLP, concat, 2-layer MLP head -> [50000, 32].

Strategy (8 NeuronCores, single SPMD launch):
- dst-node sharding: global tiles of 128 nodes; snake-dealt across cores by
  edge count so one compile-time chunk schedule serves all 8 cores.
- GCN norm factorization: agg[v] = dinv[v] * sum_{e->v} (dinv*h)[src_e]; the
  per-edge norm disappears by storing h~ = dinv*h in the gather table.
- layer-0 trick: GCNConv's weight matmul commutes with the aggregation, so
  the layer-0 gather table is just x~ = dinv*x (uploaded directly from the
  host, no device compute and no AllGather); W0 is applied per-slot AFTER
  aggregation. Layers 1/2 tables are AllGathered per 7-slot block from the
  production epilogues (overlapped with compute).
- TRANSPOSED aggregation: psT[feat, dst] += glo_chunk.T @ onehot_chunk, so
  the epilogue produces h^T directly (no PE transposes anywhere), the layer
  bias is a native per-partition activation bias, and dinv[dst] is applied
  with one DVE multiply against a host-precomputed broadcast plane.
- gathers: ONE large bf16 dma_gather per (4-slot group x table-half)
  (int16 gather indices address at most 32768 rows -> table split at row
  32768), prefetched 2 groups ahead, round-robin over 4 SWDGE queues.
- question path: qq = relu(qe@fc0+b)@fc1[128:] kept on-chip; added in the
  head via a one-hot-over-graphs matmul (sel plane precomputed on host).
- head: u^T = fc1a.T @ h3^T + qq.T @ sel; out = relu(u)^T @ fc2 + fc2_b.

Host preprocessing is index work only (sharding, edge sort, index planes);
all O(E*F) / O(N*F*F) float work runs on device.
"""
import os
import sys
import types
from contextlib import ExitStack

import numpy as np

# ---------------------------------------------------------------- constants
N = 50000
E = 800000
G = 64
P = 128
NCORES = 8
TPC = 49  # tile slots per core
SPB = 7  # slots per AllGather block
NBLK = 7
SLOT_ROWS = TPC * P  # 6272
NT = NCORES * SLOT_ROWS  # 50176
NPOS = NT // P  # 392 global table tile positions
HALF = 32768  # int16 gather index limit -> table split row
QD = 768
OUTC = 32
GROUP = 4  # slots per gather group / head group

BF16 = np.dtype("bfloat16")


def _install_axon_prof():
    """Register NTFF profile hook if the image's antenv lacks it; neuter
    bucket upload (zero-egress). Harmless when running without tracing."""
    try:
        from antenv import axon_hooks  # noqa: F401
    except ImportError:
        try:
            import antenv
            from trn_agent_boot.trn_boot import _ntff_profile_via_ctypes

            hook = _ntff_profile_via_ctypes("/opt/axon/libaxon_pjrt.so")
            mod = types.ModuleType("antenv.axon_hooks")
            mod.get_axon_ntff_profile_hook = lambda: hook
            mod.set_axon_ntff_profile_hook = lambda h: None
            sys.modules["antenv.axon_hooks"] = mod
            antenv.axon_hooks = mod
        except Exception:
            pass
    try:
        import concourse.bass_utils as bu

        bu.upload_artifacts = lambda tmpdir: "local://" + str(tmpdir)
    except Exception:
        pass


def _wrap16(arr):
    """int array -> [128, len/16] int16 plane (idx i at partition i%16,
    col i//16; replicated to all 8 gpsimd core groups)."""
    m = np.asarray(arr, dtype=np.int16).reshape(-1, 16).T
    return np.tile(m, (8, 1))


# ---------------------------------------------------------------- host prep
def preprocess(edge_index, batch):
    src = np.asarray(edge_index[0], dtype=np.int64)
    dst = np.asarray(edge_index[1], dtype=np.int64)
    deg = (np.bincount(dst, minlength=N) + 1).astype(np.float64)
    dinv = (1.0 / np.sqrt(deg)).astype(np.float32)

    n_tiles = (N + P - 1) // P  # 391
    tile_of_node = np.arange(N) // P
    dst_tile = dst // P
    tile_counts = np.bincount(dst_tile, minlength=n_tiles)

    # snake-deal tiles (sorted by edge count desc) across cores
    order_all = np.argsort(-tile_counts, kind="stable")
    core_tiles = [[] for _ in range(NCORES)]
    for r in range(TPC):
        batch_t = order_all[r * NCORES : (r + 1) * NCORES]
        seq = range(NCORES) if r % 2 == 0 else range(NCORES - 1, -1, -1)
        for j, c in enumerate(seq):
            core_tiles[c].append(int(batch_t[j]) if j < len(batch_t) else -1)

    core_of_tile = np.full(n_tiles, -1, dtype=np.int64)
    slot_of_tile = np.full(n_tiles, -1, dtype=np.int64)
    for c in range(NCORES):
        for s, t in enumerate(core_tiles[c]):
            if t >= 0:
                core_of_tile[t] = c
                slot_of_tile[t] = s

    # block-major table row for every node (same layout for all 3 layers)
    blk = slot_of_tile[tile_of_node] // SPB
    table_row = (
        blk * (NCORES * SPB * P)
        + core_of_tile[tile_of_node] * (SPB * P)
        + (slot_of_tile[tile_of_node] % SPB) * P
        + (np.arange(N) % P)
    )

    order = np.argsort(dst_tile, kind="stable")
    src_sorted = src[order]
    dst_sorted = dst[order]
    sorted_tiles = dst_tile[order]
    tile_starts = np.searchsorted(sorted_tiles, np.arange(n_tiles))
    tile_ends = np.searchsorted(sorted_tiles, np.arange(n_tiles), side="right")

    src_rows = table_row[src_sorted]
    is_lo = src_rows < HALF

    # per-(core, slot, half) edge lists + common chunk schedule
    cnt = np.zeros((NCORES, TPC, 2), dtype=np.int64)
    elists = [[None] * TPC for _ in range(NCORES)]  # (rows_lo, din_lo, rows_hi, din_hi)
    for c in range(NCORES):
        for s in range(TPC):
            t = core_tiles[c][s]
            if t < 0:
                elists[c][s] = (
                    np.zeros(0, np.int64), np.zeros(0, np.int64),
                    np.zeros(0, np.int64), np.zeros(0, np.int64),
                )
                continue
            lo_, hi_ = tile_starts[t], tile_ends[t]
            rows = src_rows[lo_:hi_]
            din = dst_sorted[lo_:hi_] % P
            m = is_lo[lo_:hi_]
            elists[c][s] = (rows[m], din[m], rows[~m] - HALF, din[~m])
            cnt[c, s, 0] = int(m.sum())
            cnt[c, s, 1] = int((~m).sum())

    nch = np.ceil(cnt.max(axis=0) / P).astype(np.int64)  # [TPC, 2]

    # gather-call schedule: chunk columns [grp lo (slot-major) | grp hi]
    groups = [list(range(g, min(g + GROUP, TPC))) for g in range(0, TPC, GROUP)]
    slot_base = np.zeros((TPC, 2), dtype=np.int64)  # gather-col base per slot
    grp_info = []  # (slots, col0, lo_tot, hi_tot)
    cur = 0
    for gs in groups:
        col0 = cur
        for s in gs:
            slot_base[s, 0] = cur
            cur += nch[s, 0]
        for s in gs:
            slot_base[s, 1] = cur
            cur += nch[s, 1]
        lo_tot = int(nch[gs, 0].sum())
        hi_tot = int(nch[gs, 1].sum())
        grp_info.append((gs, col0, lo_tot, hi_tot))
    TCH = int(cur)

    # dstin plane is SLOT-major: [slot: lo chunks | hi chunks] so one DVE
    # is_equal per slot builds the whole one-hot for that slot.
    dcol = np.zeros((TPC, 2), dtype=np.int64)
    cur = 0
    for s in range(TPC):
        dcol[s, 0] = cur
        cur += nch[s, 0]
        dcol[s, 1] = cur
        cur += nch[s, 1]
    assert cur == TCH

    idx_T = np.zeros((NCORES, 128, TCH * 8), dtype=np.int16)
    dstin_T = np.full((NCORES, 128, TCH), -1.0, dtype=np.float32)
    for c in range(NCORES):
        for s in range(TPC):
            rows_lo, din_lo, rows_hi, din_hi = elists[c][s]
            for h, (rows, din) in enumerate(((rows_lo, din_lo), (rows_hi, din_hi))):
                nchunks = int(nch[s, h])
                if nchunks == 0:
                    continue
                pad = nchunks * P
                rbuf = np.zeros(pad, dtype=np.int64)
                rbuf[: len(rows)] = rows
                dbuf = np.full(pad, -1.0, dtype=np.float32)
                dbuf[: len(din)] = din.astype(np.float32)
                b = int(slot_base[s, h])
                idx_T[c, :, b * 8 : (b + nchunks) * 8] = _wrap16(rbuf)
                db = int(dcol[s, h])
                dstin_T[c, :, db : db + nchunks] = dbuf.reshape(nchunks, P).T
    del elists

    # per-core planes: dinv per slot (per-partition), dinv broadcast across
    # partitions (free-dim scale in transposed epilogue), graph-select
    # one-hot, node permutation
    dinv_slot = np.zeros((NCORES, P, TPC), dtype=np.float32)
    dinvB = np.zeros((NCORES, P, SLOT_ROWS), dtype=np.float32)
    sel = np.zeros((NCORES, G, SLOT_ROWS), dtype=np.float32)
    node_perm = np.full((NCORES, SLOT_ROWS), -1, dtype=np.int64)
    batch = np.asarray(batch, dtype=np.int64)
    for c in range(NCORES):
        for s in range(TPC):
            t = core_tiles[c][s]
            if t < 0:
                continue
            v0 = t * P
            v1 = min(v0 + P, N)
            n = v1 - v0
            dinv_slot[c, :n, s] = dinv[v0:v1]
            dinvB[c, :, s * P : s * P + n] = dinv[v0:v1][None, :]
            sel[c, batch[v0:v1], s * P + np.arange(n)] = 1.0
            node_perm[c, s * P : s * P + n] = np.arange(v0, v1)

    # layer-0 table is x~ = dinv*x in table-row order (host-assembled)
    pos_node = np.full(NT, -1, dtype=np.int64)
    pos_node[table_row] = np.arange(N)

    return dict(
        dinv=dinv,
        nch=nch,
        dcol=dcol,
        grp_info=grp_info,
        slot_base=slot_base,
        TCH=TCH,
        idx_T=idx_T,
        dstin_T=dstin_T,
        dinv_slot=dinv_slot,
        dinvB=dinvB,
        sel=sel,
        node_perm=node_perm,
        pos_node=pos_node,
    )


# ------------------------------------------------------------- bass program
def build_program(nch, dcol, grp_info, slot_base, TCH):
    import concourse.bacc as bacc
    import concourse.bass as bass
    import concourse.tile as tile
    from concourse import library_config, mybir
    from concourse.masks import make_identity

    FDT = mybir.dt.bfloat16
    F32 = mybir.dt.float32
    I16 = mybir.dt.int16

    LOMAX = max(lo for _, _, lo, _ in grp_info)
    HIMAX = max(hi for _, _, _, hi in grp_info)
    NTOTMAX = int((nch[:, 0] + nch[:, 1]).max())

    nc = bacc.Bacc("TRN2", target_bir_lowering=False, num_swdge_queues=4)
    dp = nc.declare_dram_parameter
    table0 = dp("table0", [NT, P], FDT, isOutput=False)  # x~ rows, table order
    xN_own = dp("xN_own", [P, SLOT_ROWS], FDT, isOutput=False)  # x~ own, node-major
    idx_in = dp("idx_in", [P, TCH * 8], I16, isOutput=False)
    dstin = dp("dstin", [P, TCH], FDT, isOutput=False)
    iota_in = dp("iota_in", [P, NTOTMAX * P], FDT, isOutput=False)
    dinv_in = dp("dinv_in", [P, TPC], F32, isOutput=False)
    dinvB_in = dp("dinvB_in", [P, SLOT_ROWS], F32, isOutput=False)
    sel_in = dp("sel_in", [G, SLOT_ROWS], FDT, isOutput=False)
    W_in = [dp(f"W{i}", [P, P], FDT, isOutput=False) for i in range(3)]
    brow_in = dp("brow", [P, 3], F32, isOutput=False)
    qeT_in = dp("qeT", [QD, G], F32, isOutput=False)
    fc0w_in = dp("fc0w", [QD, P], F32, isOutput=False)
    fc0bb_in = dp("fc0bb", [P, P], F32, isOutput=False)
    fc1a_in = dp("fc1a", [P, P], FDT, isOutput=False)
    fc1b_in = dp("fc1b", [P, P], F32, isOutput=False)
    fc1bcol_in = dp("fc1bcol", [P, 1], F32, isOutput=False)
    fc2w_in = dp("fc2w", [P, OUTC], FDT, isOutput=False)
    fc2bb_in = dp("fc2bb", [P, OUTC], F32, isOutput=False)
    out_d = dp("out", [SLOT_ROWS, OUTC], F32, isOutput=True)

    cc_in = nc.dram_tensor("cc_in", [SLOT_ROWS, P], FDT)
    tables = [
        table0,
        nc.dram_tensor("table1", [NT, P], FDT, addr_space="Shared"),
        nc.dram_tensor("table2", [NT, P], FDT, addr_space="Shared"),
    ]

    with tile.TileContext(nc) as tc, ExitStack() as ctx:
        nc.gpsimd.load_library(library_config.mlp)

        const = ctx.enter_context(tc.tile_pool(name="const", bufs=1))
        gp = ctx.enter_context(tc.tile_pool(name="gp", bufs=4))
        ghp = ctx.enter_context(tc.tile_pool(name="ghp", bufs=4))
        ohp = ctx.enter_context(tc.tile_pool(name="ohp", bufs=4))
        htp = ctx.enter_context(tc.tile_pool(name="htp", bufs=3))
        psagg = ctx.enter_context(tc.tile_pool(name="psagg", bufs=3, space="PSUM"))
        psp = ctx.enter_context(tc.tile_pool(name="psp", bufs=3, space="PSUM"))
        psh = ctx.enter_context(tc.tile_pool(name="psh", bufs=2, space="PSUM"))
        epi = ctx.enter_context(tc.tile_pool(name="epi", bufs=4))

        # ---- constants
        idx_sb = const.tile([P, TCH * 8], I16)
        nc.scalar.dma_start(out=idx_sb[:], in_=idx_in[:])
        dstin_sb = const.tile([P, TCH], FDT)
        nc.scalar.dma_start(out=dstin_sb[:], in_=dstin[:])
        iota_sb = const.tile([P, NTOTMAX * P], FDT)
        nc.scalar.dma_start(out=iota_sb[:], in_=iota_in[:])
        dinv_sb = const.tile([P, TPC], F32)
        nc.sync.dma_start(out=dinv_sb[:], in_=dinv_in[:])
        dinvB_sb = const.tile([P, SLOT_ROWS], F32)
        nc.sync.dma_start(out=dinvB_sb[:], in_=dinvB_in[:])
        sel_sb = const.tile([G, SLOT_ROWS], FDT)
        nc.sync.dma_start(out=sel_sb[:], in_=sel_in[:])
        W_sb = []
        for i in range(3):
            w = const.tile([P, P], FDT, tag=f"W{i}")
            nc.sync.dma_start(out=w[:], in_=W_in[i][:])
            W_sb.append(w)
        brow_sb = const.tile([P, 3], F32)
        nc.sync.dma_start(out=brow_sb[:], in_=brow_in[:])
        fc1a_sb = const.tile([P, P], FDT)
        nc.sync.dma_start(out=fc1a_sb[:], in_=fc1a_in[:])
        fc1bcol_sb = const.tile([P, 1], F32)
        nc.sync.dma_start(out=fc1bcol_sb[:], in_=fc1bcol_in[:])
        fc2w_sb = const.tile([P, OUTC], FDT)
        nc.sync.dma_start(out=fc2w_sb[:], in_=fc2w_in[:])
        fc2bb_sb = const.tile([P, OUTC], F32)
        nc.sync.dma_start(out=fc2bb_sb[:], in_=fc2bb_in[:])
        ident = const.tile([P, P], F32)
        make_identity(nc, ident[:])
        ident_r = const.tile([P, P], FDT, tag="ident_r")
        nc.vector.tensor_copy(out=ident_r[:], in_=ident[:])

        # ---- question path: qq = relu(qe@fc0+fc0_b)@fc1b  (bf16, on-chip;
        # fc1_b enters later as the head activation bias)
        qe_sb = const.tile([P, 6 * G], F32)
        fc0w_sb = const.tile([P, 6 * P], F32)
        for k in range(6):
            nc.sync.dma_start(
                out=qe_sb[:, k * G : (k + 1) * G], in_=qeT_in[k * P : (k + 1) * P, :]
            )
            nc.sync.dma_start(
                out=fc0w_sb[:, k * P : (k + 1) * P],
                in_=fc0w_in[k * P : (k + 1) * P, :],
            )
        fc0bb_sb = const.tile([P, P], F32)
        nc.sync.dma_start(out=fc0bb_sb[:], in_=fc0bb_in[:])
        fc1b_sb = const.tile([P, P], F32)
        nc.sync.dma_start(out=fc1b_sb[:], in_=fc1b_in[:])

        pq = psp.tile([G, P], F32, space="PSUM", tag="mm")
        for k in range(6):
            nc.tensor.matmul(
                out=pq[:],
                lhsT=qe_sb[:, k * G : (k + 1) * G],
                rhs=fc0w_sb[:, k * P : (k + 1) * P],
                start=(k == 0),
                stop=(k == 5),
            )
        qtmp = epi.tile([G, P], F32, tag="qtmp")
        nc.vector.tensor_tensor(
            out=qtmp[:], in0=pq[:], in1=fc0bb_sb[:G, :], op=mybir.AluOpType.add
        )
        qrelu = epi.tile([G, P], F32, tag="qrelu")
        nc.scalar.activation(
            out=qrelu[:], in_=qtmp[:], func=mybir.ActivationFunctionType.Relu
        )
        pqt = psh.tile([P, G], F32, space="PSUM", tag="hd")
        nc.tensor.transpose(out=pqt[:], in_=qrelu[:], identity=ident[:G, :G])
        qT = epi.tile([P, G], F32, tag="qT")
        nc.scalar.copy(out=qT[:], in_=pqt[:])
        pqq = psp.tile([G, P], F32, space="PSUM", tag="mm")
        nc.tensor.matmul(
            out=pqq[:], lhsT=qT[:], rhs=fc1b_sb[:], start=True, stop=True
        )
        qq_sb = const.tile([G, P], FDT, tag="qq_sb")
        nc.vector.tensor_copy(out=qq_sb[:], in_=pqq[:])

        # resident own-slice h~ buffers (self-loop term source), layer parity.
        # hs_keep[0] starts as x~ own slots (node-major) = layer-0 self terms.
        hs_keep = [
            const.tile([P, SLOT_ROWS], FDT, tag=f"hsk{i}", name=f"hsk{i}")
            for i in range(2)
        ]
        nc.sync.dma_start(out=hs_keep[0][:], in_=xN_own[:])

        def allgather_block(l, j):
            r0 = j * SPB * P
            r1 = (j + 1) * SPB * P
            nc.gpsimd.collective_compute(
                "AllGather",
                mybir.AluOpType.bypass,
                replica_groups=[list(range(NCORES))],
                ins=[cc_in[r0:r1].opt()],
                outs=[tables[l][j * NCORES * SPB * P : (j + 1) * NCORES * SPB * P].opt()],
            )

        # multi-queue gather helper: <=8-chunk single-packet calls, round-robin
        # over the 4 SWDGE queues (queues generate descriptors in parallel)
        qctr = [0]

        def gather(dst_tile, src_ap, col_base, nchunks):
            k = 0
            while k < nchunks:
                nb = min(8, nchunks - k)
                nc.gpsimd.dma_gather(
                    out_ap=dst_tile[
                        :, k * P : (k + nb) * P
                    ].rearrange("p (k q) -> p k q", q=P),
                    in_ap=src_ap,
                    idxs_ap=idx_sb[:, (col_base + k) * 8 : (col_base + k + nb) * 8],
                    num_idxs=nb * P,
                    num_idxs_reg=nb * P,
                    elem_size=P,
                    single_packet=True,
                    queue_num=qctr[0] % 4,
                )
                qctr[0] += 1
                k += nb

        NG = len(grp_info)
        PF = 3  # gather prefetch depth (groups)

        # ---- 3 aggregation layers
        for l in range(3):
            table = tables[l]
            gtiles = {}

            def issue_gather(gi):
                gs, col0, lo_tot, hi_tot = grp_info[gi]
                glo = gp.tile([P, LOMAX * P], FDT, tag="glo")
                if lo_tot:
                    gather(glo, table[0:HALF], col0, lo_tot)
                ghi = ghp.tile([P, HIMAX * P], FDT, tag="ghi")
                if hi_tot:
                    gather(ghi, table[HALF:NT], col0 + lo_tot, hi_tot)
                gtiles[gi] = (glo, ghi)

            for gi in range(min(PF, NG)):
                issue_gather(gi)

            for gi in range(NG):
                if gi + PF < NG:
                    issue_gather(gi + PF)
                gs, col0, lo_tot, hi_tot = grp_info[gi]
                glo, ghi = gtiles.pop(gi)
                ncols = len(gs) * P
                hT = htp.tile([P, GROUP * P], FDT, tag="hT")
                psg = psagg.tile([P, GROUP * P], F32, space="PSUM", tag="agg")
                for si, s in enumerate(gs):
                    nlo = int(nch[s, 0])
                    nhi = int(nch[s, 1])
                    ntot = nlo + nhi
                    lo_rel = int(slot_base[s, 0]) - col0  # within glo
                    hi_rel = int(slot_base[s, 1]) - col0 - lo_tot  # within ghi
                    db = int(dcol[s, 0])
                    # one-hot for this slot's chunks (lo then hi)
                    oh = ohp.tile([P, NTOTMAX * P], FDT, tag="oh")
                    if ntot:
                        nc.vector.tensor_tensor(
                            out=oh[:, : ntot * P].rearrange("p (k q) -> p k q", q=P),
                            in0=iota_sb[:, : ntot * P].rearrange(
                                "p (k q) -> p k q", q=P
                            ),
                            in1=dstin_sb[:, db : db + ntot].to_broadcast(
                                [P, ntot, P]
                            ),
                            op=mybir.AluOpType.is_equal,
                        )
                    # transposed aggregation: psT[feat, dst]
                    ps = psg[:, si * P : (si + 1) * P]
                    for k in range(nlo):
                        nc.tensor.matmul(
                            out=ps,
                            lhsT=glo[:, (lo_rel + k) * P : (lo_rel + k + 1) * P],
                            rhs=oh[:, k * P : (k + 1) * P],
                            start=(k == 0),
                            stop=False,
                        )
                    for k in range(nhi):
                        nc.tensor.matmul(
                            out=ps,
                            lhsT=ghi[:, (hi_rel + k) * P : (hi_rel + k + 1) * P],
                            rhs=oh[:, (nlo + k) * P : (nlo + k + 1) * P],
                            start=False,
                            stop=False,
                        )
                    # self-loop: psT += h~_slot^T
                    nc.tensor.matmul(
                        out=ps,
                        lhsT=hs_keep[l % 2][:, s * P : (s + 1) * P],
                        rhs=ident_r[:],
                        start=(ntot == 0),
                        stop=True,
                    )
                    if l == 0:
                        # layer 0 aggregated x~; apply W0 now (it commutes
                        # with the sum): psW = W0^T @ aggx
                        aggx = epi.tile([P, P], FDT, tag="aggx")
                        nc.scalar.activation(
                            out=aggx[:],
                            in_=ps,
                            func=mybir.ActivationFunctionType.Copy,
                        )
                        psW = psp.tile([P, P], F32, space="PSUM", tag="mm")
                        nc.tensor.matmul(
                            out=psW[:], lhsT=W_sb[0][:], rhs=aggx[:],
                            start=True, stop=True,
                        )
                        ps = psW[:]
                    # epilogue: hT = relu(dinv[dst] * psT + b_l)
                    tmp = epi.tile([P, P], F32, tag="tmp")
                    nc.vector.tensor_tensor(
                        out=tmp[:],
                        in0=ps,
                        in1=dinvB_sb[:, s * P : (s + 1) * P],
                        op=mybir.AluOpType.mult,
                    )
                    nc.scalar.activation(
                        out=hT[:, si * P : (si + 1) * P],
                        in_=tmp[:],
                        func=mybir.ActivationFunctionType.Relu,
                        bias=brow_sb[:, l : l + 1],
                    )
                    if l < 2:
                        # next-layer production: h~ = dinv * (h @ W)
                        pp2 = psp.tile([P, P], F32, space="PSUM", tag="mm")
                        nc.tensor.matmul(
                            out=pp2[:],
                            lhsT=hT[:, si * P : (si + 1) * P],
                            rhs=W_sb[l + 1][:],
                            start=True,
                            stop=True,
                        )
                        hs2 = hs_keep[(l + 1) % 2][:, s * P : (s + 1) * P]
                        nc.scalar.activation(
                            out=hs2,
                            in_=pp2[:],
                            func=mybir.ActivationFunctionType.Copy,
                            scale=dinv_sb[:, s : s + 1],
                        )
                        nc.sync.dma_start(
                            out=cc_in[s * P : (s + 1) * P, :], in_=hs2
                        )
                        if (s + 1) % SPB == 0:
                            allgather_block(l + 1, s // SPB)
                if l == 2:
                    # MLP head (transposed): uT = fc1a^T h3T + qq^T sel
                    s0 = gs[0]
                    pu = psh.tile([P, GROUP * P], F32, space="PSUM", tag="hd")
                    nc.tensor.matmul(
                        out=pu[:, :ncols],
                        lhsT=fc1a_sb[:],
                        rhs=hT[:, :ncols],
                        start=True,
                        stop=False,
                    )
                    nc.tensor.matmul(
                        out=pu[:, :ncols],
                        lhsT=qq_sb[:],
                        rhs=sel_sb[:, s0 * P : s0 * P + ncols],
                        start=False,
                        stop=True,
                    )
                    ur = epi.tile([P, GROUP * P], FDT, tag="ur")
                    nc.scalar.activation(
                        out=ur[:, :ncols],
                        in_=pu[:, :ncols],
                        func=mybir.ActivationFunctionType.Relu,
                        bias=fc1bcol_sb[:],
                    )
                    for si, s in enumerate(gs):
                        po = psp.tile([P, OUTC], F32, space="PSUM", tag="mm")
                        nc.tensor.matmul(
                            out=po[:],
                            lhsT=ur[:, si * P : (si + 1) * P],
                            rhs=fc2w_sb[:],
                            start=True,
                            stop=True,
                        )
                        ob = epi.tile([P, OUTC], F32, tag="ob")
                        nc.vector.tensor_tensor(
                            out=ob[:],
                            in0=po[:],
                            in1=fc2bb_sb[:],
                            op=mybir.AluOpType.add,
                        )
                        nc.sync.dma_start(
                            out=out_d[s * P : (s + 1) * P, :], in_=ob[:]
                        )
    nc.compile()
    return nc


# ---------------------------------------------------------------- interface
_CACHE = {}


def kernel(**inputs):
    trace = bool(int(os.environ.get("GCN_TRACE", "0")))
    if trace:
        _install_axon_prof()
    from concourse.bass_utils import run_bass_kernel_spmd

    x = np.ascontiguousarray(np.asarray(inputs["x"], dtype=np.float32))
    qe = np.asarray(inputs["question_embedding"], dtype=np.float32)
    pp = preprocess(inputs["edge_index"], inputs["batch"])
    nch = pp["nch"]

    key = tuple(nch.flatten().tolist())
    if key not in _CACHE:
        _CACHE[key] = build_program(
            nch, pp["dcol"], pp["grp_info"], pp["slot_base"], pp["TCH"]
        )
    nc = _CACHE[key]

    NTOTMAX = int((nch[:, 0] + nch[:, 1]).max())

    W = [np.asarray(inputs[f"W{i}"], np.float32) for i in range(3)]
    b = [np.asarray(inputs[f"b{i}"], np.float32) for i in range(3)]
    fc0_w = np.asarray(inputs["fc0_w"], np.float32)
    fc0_b = np.asarray(inputs["fc0_b"], np.float32)
    fc1_w = np.asarray(inputs["fc1_w"], np.float32)
    fc1_b = np.asarray(inputs["fc1_b"], np.float32)
    fc2_w = np.asarray(inputs["fc2_w"], np.float32)
    fc2_b = np.asarray(inputs["fc2_b"], np.float32)

    # layer-0 gather table: x~ = dinv*x rows in table order
    xs = pp["dinv"][:, None] * x
    table0 = np.zeros((NT, P), dtype=BF16)
    valid = pp["pos_node"] >= 0
    table0[valid] = xs[pp["pos_node"][valid]].astype(BF16)

    iota = np.broadcast_to(np.arange(P, dtype=np.float32), (P, P))
    iota_rep = np.ascontiguousarray(np.tile(iota, (1, NTOTMAX)).astype(BF16))
    common = {
        "table0": table0,
        "iota_in": iota_rep,
        "W0": W[0].astype(BF16),
        "W1": W[1].astype(BF16),
        "W2": W[2].astype(BF16),
        "brow": np.stack(b, axis=1).astype(np.float32).copy(),
        "qeT": np.ascontiguousarray(qe.T),
        "fc0w": fc0_w,
        "fc0bb": np.broadcast_to(fc0_b, (P, P)).copy(),
        "fc1a": np.ascontiguousarray(fc1_w[:P]).astype(BF16),
        "fc1b": np.ascontiguousarray(fc1_w[P:]),
        "fc1bcol": fc1_b.reshape(P, 1).copy(),
        "fc2w": fc2_w.astype(BF16),
        "fc2bb": np.broadcast_to(fc2_b, (P, OUTC)).copy(),
    }

    in_maps = []
    for c in range(NCORES):
        # x~ own slots, node-major: column s*P+i holds... rows=node within
        # slot on partitions, feature along free dim per slot block
        xNc = np.zeros((P, SLOT_ROWS), dtype=BF16)
        validc = pp["node_perm"][c] >= 0
        xs_own = np.zeros((SLOT_ROWS, P), dtype=np.float32)
        xs_own[validc] = xs[pp["node_perm"][c][validc]]
        # slot block s: tile [node, feat] -> xN[node_part, s*P + feat]
        xNc[:, :] = np.hstack([xs_own[s * P : (s + 1) * P, :] for s in range(TPC)]).astype(BF16)
        m = dict(common)
        m["xN_own"] = xNc
        m["idx_in"] = np.ascontiguousarray(pp["idx_T"][c])
        m["dstin"] = np.ascontiguousarray(pp["dstin_T"][c].astype(BF16))
        m["dinv_in"] = np.ascontiguousarray(pp["dinv_slot"][c])
        m["dinvB_in"] = np.ascontiguousarray(pp["dinvB"][c])
        m["sel_in"] = np.ascontiguousarray(pp["sel"][c].astype(BF16))
        in_maps.append(m)

    res = run_bass_kernel_spmd(
        nc,
        in_maps,
        list(range(NCORES)),
        trace=trace,
    )
    kernel.last_result = res

    out = np.zeros((N, OUTC), dtype=np.float32)
    for c in range(NCORES):
        validc = pp["node_perm"][c] >= 0
        out[pp["node_perm"][c][validc]] = res.results[c]["out"][validc]
    return out
